# revision 4
# baseline (speedup 1.0000x reference)
"""ConcatAttention Trainium2 Bass kernel — 8-core data-parallel.

Reference (per batch b):
    precompute = context @ W_pre.T + b_pre          # [S, A]   (output)
    targetT    = input @ W_q.T                      # [A]
    tmp        = tanh(precompute + targetT)         # [S, A]
    energy     = tmp @ v                            # [S]
    score      = softmax(energy)                    # [S]      (output)
    wc         = score @ context                    # [AD]     (output)

Shapes: B=64, S=2048, AD=QD=A=1024, fp32. Batch is sharded 8-ways
(8 batches per core); weights replicated.

Device strategy (per core):
 - All matmuls run as float32r (full fp32 operands, ~4x faster than
   fp32 matmul mode on the PE; measured max rel err ~1.4e-4).
 - The big matmul computes precomputeT [A, S] (a on partitions) so the
   per-a bias (b_pre) and per-(a,b) bias (b_pre + targetT) are
   per-partition scalars: the bias+tanh fuses into one ScalarE
   activation straight out of PSUM.
 - context is supplied in BOTH layouts from the host: transposed
   [AD, S] for the big matmul (the PE contracts over the partition dim,
   so d must sit on partitions) and natural [S, AD] for the
   score-weighted sum. Host-side transposes are layout prep only.
 - energy = sum_a v[a] * tmpT[a, s] uses a masked lhsT ([128, 8] with
   v in column b, zeros elsewhere) so all 8 batches accumulate into one
   PSUM tile [8, S] with batch = partition; softmax then runs batched
   on all 8 rows at once.
 - weightedContext uses the same masked-lhsT trick with the transposed
   score, accumulating into one PSUM tile [8, AD].
 - precompute is written to DRAM as [A, S] per batch and transposed
   back to [S, A] on the host (layout only).
"""

import os
import sys
import types

sys.path.insert(0, "/opt/trn_rl_repo")

import numpy as np

import concourse.bass as bass  # noqa: E402
import concourse.tile as tile  # noqa: E402
from concourse import bacc, bass_utils, mybir  # noqa: E402

B, S, AD, QD, A = 64, 2048, 1024, 1024, 1024
NCORES = 8
BL = B // NCORES  # batches per core = 8
DC = AD // 128    # d chunks = 8
ACH = A // 128    # a chunks = 8
SC = S // 128     # s chunks = 16
SH = S // 512     # s halves (512-wide) = 4
F32 = mybir.dt.float32
F32R = mybir.dt.float32r
AFT = mybir.ActivationFunctionType
AXX = mybir.AxisListType.X

_NC = None        # cached compiled Bass program
LAST_RES = None   # last BassKernelResults (test harness reads timing)


def _install_profile_hook():
    """Make bass_utils' NTFF tracing work in this image (missing
    antenv.axon_hooks). Harmless if tracing is never requested."""
    try:
        import antenv
        if "antenv.axon_hooks" in sys.modules:
            return
        from trn_agent_boot.trn_boot import _ntff_profile_via_ctypes
        m = types.ModuleType("antenv.axon_hooks")
        hook = _ntff_profile_via_ctypes("/opt/axon/libaxon_pjrt.so")
        m.get_axon_ntff_profile_hook = lambda: hook
        m.set_axon_ntff_profile_hook = lambda h: None
        sys.modules["antenv.axon_hooks"] = m
        antenv.axon_hooks = m
    except Exception:
        pass


def _build():
    nc = bacc.Bacc("TRN2", target_bir_lowering=False, debug=False)

    ctxT_d = nc.dram_tensor("ctxT", [BL, AD, S], F32R, kind="ExternalInput").ap()
    ctxn_d = nc.dram_tensor("ctxn", [BL, S, AD], F32R, kind="ExternalInput").ap()
    wpreT_d = nc.dram_tensor("wpreT", [AD, A], F32R, kind="ExternalInput").ap()
    wqT_d = nc.dram_tensor("wqT", [QD, A], F32R, kind="ExternalInput").ap()
    inT_d = nc.dram_tensor("inT", [QD, BL], F32R, kind="ExternalInput").ap()
    bpreT_d = nc.dram_tensor("bpreT", [128, ACH], F32, kind="ExternalInput").ap()
    vmask_d = nc.dram_tensor("vmask", [128, ACH, BL, BL], F32R, kind="ExternalInput").ap()
    id8_d = nc.dram_tensor("id8", [BL, BL], F32, kind="ExternalInput").ap()
    zeros_d = nc.dram_tensor("zeros", [128, SC, BL, BL], F32R, kind="ExternalInput").ap()

    precT_d = nc.dram_tensor("precT", [BL, A, S], F32, kind="ExternalOutput").ap()
    score_d = nc.dram_tensor("score", [BL, S], F32, kind="ExternalOutput").ap()
    wc_d = nc.dram_tensor("wc", [BL, AD], F32, kind="ExternalOutput").ap()

    with tile.TileContext(nc) as tc:
        with (
            tc.tile_pool(name="big", bufs=2) as big,
            tc.tile_pool(name="wpre", bufs=1) as wpool,
            tc.tile_pool(name="prec", bufs=2) as prec_p,
            tc.tile_pool(name="tmp", bufs=4) as tmp_p,
            tc.tile_pool(name="small", bufs=1) as small,
            tc.tile_pool(name="soft", bufs=2) as soft,
            tc.tile_pool(name="psmm", bufs=2, space="PSUM") as ps_mm,
            tc.tile_pool(name="psen", bufs=1, space="PSUM") as ps_en,
            tc.tile_pool(name="psmisc", bufs=1, space="PSUM") as ps_misc,
        ):
            # ---- setup loads ----
            wpre_sb = wpool.tile([128, DC, A], F32R)
            nc.sync.dma_start(wpre_sb[:], wpreT_d.rearrange("(c p) a -> p c a", p=128))
            wq_sb = big.tile([128, DC, A], F32R, tag="big")
            nc.sync.dma_start(wq_sb[:], wqT_d.rearrange("(c p) a -> p c a", p=128))
            int_sb = small.tile([128, DC, BL], F32R)
            nc.sync.dma_start(int_sb[:], inT_d.rearrange("(c p) b -> p c b", p=128))
            bpre_sb = small.tile([128, ACH], F32)
            nc.sync.dma_start(bpre_sb[:], bpreT_d)
            vm_sb = small.tile([128, ACH, BL, BL], F32R)
            nc.sync.dma_start(vm_sb[:], vmask_d)
            id8_sb = small.tile([BL, BL], F32)
            nc.sync.dma_start(id8_sb[:], id8_d)

            # ---- targetT^T + fused tanh bias:  bias2[a, b] = b_pre[a] + (W_q @ input[b])[a]
            bias2_sb = small.tile([128, ACH, BL], F32)
            for ac in range(ACH):
                ptt = ps_misc.tile([128, BL], F32, tag="misc")
                for qc in range(DC):
                    nc.tensor.matmul(
                        ptt[:],
                        wq_sb[:, qc, ac * 128:(ac + 1) * 128],
                        int_sb[:, qc, :],
                        start=(qc == 0), stop=(qc == DC - 1),
                    )
                nc.vector.tensor_scalar_add(bias2_sb[:, ac, :], ptt[:], bpre_sb[:, ac:ac + 1])

            # ---- phase 1: precomputeT, tanh, energy ----
            en_ps = ps_en.tile([BL, S], F32)
            for b in range(BL):
                ctxt_sb = big.tile([128, DC, S], F32R, tag="big")
                nc.sync.dma_start(ctxt_sb[:], ctxT_d[b].rearrange("(c p) s -> p c s", p=128))
                for ac in range(ACH):
                    prec_sb = prec_p.tile([128, S], F32)
                    tmps = []
                    for sh in range(SH):
                        sl = slice(sh * 512, (sh + 1) * 512)
                        pmm = ps_mm.tile([128, 512], F32)
                        for dc in range(DC):
                            nc.tensor.matmul(
                                pmm[:],
                                wpre_sb[:, dc, ac * 128:(ac + 1) * 128],
                                ctxt_sb[:, dc, sl],
                                start=(dc == 0), stop=(dc == DC - 1),
                            )
                        nc.vector.tensor_scalar_add(prec_sb[:, sl], pmm[:], bpre_sb[:, ac:ac + 1])
                        tt = tmp_p.tile([128, 512], F32R)
                        nc.scalar.activation(tt[:], pmm[:], AFT.Tanh, bias=bias2_sb[:, ac, b:b + 1])
                        tmps.append((sh, tt))
                    for sh, tt in tmps:
                        nc.tensor.matmul(
                            en_ps[:, sh * 512:(sh + 1) * 512],
                            vm_sb[:, ac, b, :],
                            tt[:],
                            start=(b == 0 and ac == 0), stop=(b == BL - 1 and ac == ACH - 1),
                        )
                    nc.sync.dma_start(precT_d[b, ac * 128:(ac + 1) * 128, :], prec_sb[:])

            # ---- softmax over S, batched on all 8 rows ----
            mx = small.tile([BL, 1], F32)
            nc.vector.reduce_max(mx[:], en_ps[:], axis=AXX)
            negmx = small.tile([BL, 1], F32)
            nc.vector.tensor_scalar_mul(negmx[:], mx[:], -1.0)
            exp_sb = soft.tile([BL, S], F32, tag="soft")
            nc.scalar.activation(exp_sb[:], en_ps[:], AFT.Exp, bias=negmx[:])
            sume = small.tile([BL, 1], F32)
            nc.vector.reduce_sum(sume[:], exp_sb[:], axis=AXX)
            rsum = small.tile([BL, 1], F32)
            nc.vector.reciprocal(rsum[:], sume[:])
            score_sb = soft.tile([BL, S], F32, tag="soft")
            nc.vector.tensor_scalar_mul(score_sb[:], exp_sb[:], rsum[:])
            nc.sync.dma_start(score_d[:], score_sb[:])

            # ---- score^T  [s -> partitions], then masked copy for wc lhsT ----
            scps = ps_misc.tile([128, SC * BL], F32, tag="misc")
            for c in range(SC):
                nc.tensor.transpose(
                    scps[:, c * BL:(c + 1) * BL],
                    score_sb[:, c * 128:(c + 1) * 128],
                    id8_sb[:],
                )
            scT = small.tile([128, SC, BL], F32)
            nc.scalar.copy(scT[:], scps[:].rearrange("p (c e) -> p c e", e=BL))
            scm = small.tile([128, SC, BL, BL], F32R)
            nc.sync.dma_start(scm[:], zeros_d)
            for b in range(BL):
                nc.vector.tensor_copy(scm[:, :, b, b], scT[:, :, b])

            # ---- phase 2: weightedContext ----
            wcps = ps_misc.tile([BL, AD], F32, tag="misc")
            for b in range(BL):
                cn = big.tile([128, SC, AD], F32R, tag="big")
                nc.sync.dma_start(cn[:], ctxn_d[b].rearrange("(so si) d -> si so d", si=128))
                for c in range(SC):
                    for nh in range(2):
                        nc.tensor.matmul(
                            wcps[:, nh * 512:(nh + 1) * 512],
                            scm[:, c, b, :],
                            cn[:, c, nh * 512:(nh + 1) * 512],
                            start=(b == 0 and c == 0), stop=(b == BL - 1 and c == SC - 1),
                        )
            wc_sb = soft.tile([BL, AD], F32, tag="soft")
            nc.scalar.copy(wc_sb[:], wcps[:])
            nc.sync.dma_start(wc_d[:], wc_sb[:])

    nc.compile()
    return nc


def kernel(input, context, W_pre, b_pre, W_q, v):
    global _NC, LAST_RES
    _install_profile_hook()

    input = np.ascontiguousarray(np.asarray(input, dtype=np.float32))
    context = np.ascontiguousarray(np.asarray(context, dtype=np.float32))
    W_pre = np.asarray(W_pre, dtype=np.float32)
    b_pre = np.asarray(b_pre, dtype=np.float32)
    W_q = np.asarray(W_q, dtype=np.float32)
    v = np.asarray(v, dtype=np.float32)

    if _NC is None:
        _NC = _build()

    ctxT_all = np.ascontiguousarray(context.transpose(0, 2, 1))  # [B, AD, S]
    wpreT = np.ascontiguousarray(W_pre.T)                        # [AD, A]
    wqT = np.ascontiguousarray(W_q.T)                            # [QD, A]
    inT_all = np.ascontiguousarray(input.T)                      # [QD, B]
    bpreT = np.ascontiguousarray(b_pre.reshape(ACH, 128).T)      # [128, ACH]
    vm = v.reshape(ACH, 128).T                                   # [128, ACH]
    vmask = np.zeros((128, ACH, BL, BL), np.float32)
    for b in range(BL):
        vmask[:, :, b, b] = vm
    id8 = np.eye(BL, dtype=np.float32)
    zeros_scm = np.zeros((128, SC, BL, BL), np.float32)

    in_maps = []
    for c in range(NCORES):
        bs = slice(c * BL, (c + 1) * BL)
        in_maps.append({
            "ctxT": ctxT_all[bs],
            "ctxn": context[bs],
            "wpreT": wpreT,
            "wqT": wqT,
            "inT": np.ascontiguousarray(inT_all[:, bs]),
            "bpreT": bpreT,
            "vmask": vmask,
            "id8": id8,
            "zeros": zeros_scm,
        })

    LAST_RES = bass_utils.run_bass_kernel_spmd(_NC, in_maps, core_ids=list(range(NCORES)))
    rs = LAST_RES.results

    prec = np.concatenate([r["precT"] for r in rs], axis=0)      # [B, A, S]
    prec = np.ascontiguousarray(prec.transpose(0, 2, 1))         # [B, S, A]
    score = np.concatenate([r["score"] for r in rs], axis=0)     # [B, S]
    wc = np.concatenate([r["wc"] for r in rs], axis=0)           # [B, AD]
    return wc, score, prec
